# revision 10
# baseline (speedup 1.0000x reference)
"""CPCLoss (CE + BDC + BEC) Trainium2 kernel.

Strategy (data-parallel over N across 8 NeuronCores, 1024 rows/core):
  Per row, the BEC term needs sum over all ordered pairs (j,k) of
  logsigmoid(x_j - x_k (+eps)).  Using the identity
      softplus(d) + softplus(-d) = d + 2*ln(1 + exp(-d))
  only the 4950 unordered pairs are evaluated; the signed-pair sum
  sum_q d_q is a fixed linear functional of the row, computed as one extra
  matmul column with weights (C-1-2i).  On device:
    - TensorE computes all pair differences d = x_j - x_k via a constant
      {+1,-1} difference matrix (bf16 hi/lo split of x for fp32-grade
      precision), accumulated in PSUM.
    - ScalarE reads PSUM directly: exp(-d), then ln(1 + u) over SBUF (both
      functions live in the single 'natural_log_exp_and_others' activation
      table set -> one table load for the whole kernel), accumulating
      sum ln(1+e^-d) per partition via accum_out.
  CE (logsumexp) and BDC plus the two BEC "target row/col" corrections are
  tiny (O(N*C)) and use the same exp/ln tables.
  Per-core partial sums land in small [128, k] tensors; the host combines
  them in float64.
"""

import math
import os
import sys

sys.path.insert(0, "/opt/trn_rl_repo")

import numpy as np
import ml_dtypes

import concourse.bacc as bacc
import concourse.tile as tile
from concourse import mybir
from concourse.bass_utils import run_bass_kernel_spmd

F32 = mybir.dt.float32
BF16 = mybir.dt.bfloat16
AF = mybir.ActivationFunctionType
ALU = mybir.AluOpType

N, C = 8192, 100
NCORES = 8
RPC = N // NCORES          # rows per core = 1024
P = 128                    # partitions
T = RPC // P               # row-tiles per core = 8
EPS = 1e-7
NPAIR = (C * (C - 1)) // 2  # 4950
NMCOL = NPAIR + 1           # extra col: sum_q d_q weights
CHUNK = 512
NCHUNK = (NMCOL + CHUNK - 1) // CHUNK   # 10 (last = 343)
# psum groups: chunks [0..3], [4..7], [8..9]
GROUPS = [(0, 4), (4, 4), (8, 2)]

_PAIR_J, _PAIR_K = np.triu_indices(C, 1)

_cache = {}


def _build_module():
    nc = bacc.Bacc("TRN2", target_bir_lowering=False, debug=False)

    xrow_d = nc.dram_tensor("xrow", [P, T, C], F32, kind="ExternalInput")
    xthi_d = nc.dram_tensor("xthi", [C, RPC], BF16, kind="ExternalInput")
    xtlo_d = nc.dram_tensor("xtlo", [C, RPC], BF16, kind="ExternalInput")
    mmat_d = nc.dram_tensor("mmat", [C, NMCOL], BF16, kind="ExternalInput")
    xy_d = nc.dram_tensor("xy", [P, T], F32, kind="ExternalInput")
    xyea_d = nc.dram_tensor("xy_eps_a", [P, T], F32, kind="ExternalInput")
    xyme_d = nc.dram_tensor("xy_meps", [P, T], F32, kind="ExternalInput")

    sumd_d = nc.dram_tensor("sumd", [P, T], F32, kind="ExternalOutput")
    sumln_d = nc.dram_tensor("sumln", [P, T], F32, kind="ExternalOutput")
    abt_d = nc.dram_tensor("abt", [P, 3], F32, kind="ExternalOutput")
    lnse_d = nc.dram_tensor("lnse", [P, T], F32, kind="ExternalOutput")
    mrow_d = nc.dram_tensor("mrow", [P, T], F32, kind="ExternalOutput")

    with tile.TileContext(nc) as tc:
        with (
            tc.tile_pool(name="consts", bufs=1) as consts,
            tc.tile_pool(name="work", bufs=2) as work,
            tc.tile_pool(name="psum", bufs=2, space="PSUM") as psum,
        ):
            # ---- load inputs ----
            xrow = consts.tile([P, T, C], F32)
            nc.sync.dma_start(out=xrow[:], in_=xrow_d[:])
            xthi = consts.tile([C, RPC], BF16)
            nc.sync.dma_start(out=xthi[:], in_=xthi_d[:])
            xtlo = consts.tile([C, RPC], BF16)
            nc.sync.dma_start(out=xtlo[:], in_=xtlo_d[:])
            xy = consts.tile([P, T], F32)
            nc.sync.dma_start(out=xy[:], in_=xy_d[:])
            xyea = consts.tile([P, T], F32)
            nc.sync.dma_start(out=xyea[:], in_=xyea_d[:])
            xyme = consts.tile([P, T], F32)
            nc.sync.dma_start(out=xyme[:], in_=xyme_d[:])
            # difference matrix, chunked DMA so early matmuls start sooner
            msb = consts.tile([C, NMCOL], BF16)
            for ci in range(NCHUNK):
                q0 = ci * CHUNK
                w = min(CHUNK, NMCOL - q0)
                nc.sync.dma_start(out=msb[:, q0:q0 + w], in_=mmat_d[:, q0:q0 + w])

            # ---- accumulators ----
            sumd = consts.tile([P, T], F32)
            sumln = consts.tile([P, T], F32)
            abt = consts.tile([P, 3], F32)
            lnse = consts.tile([P, T], F32)
            mrow = consts.tile([P, T], F32)
            se = consts.tile([P, T], F32)
            zce = consts.tile([P, T, C], F32)
            zbdc = consts.tile([P, T, C], F32)
            za = consts.tile([P, T, C], F32)
            zb = consts.tile([P, T, C], F32)

            # ---- CE prep: row max then x - m (per tile; scalar1 is per-partition) ----
            nc.vector.tensor_reduce(
                out=mrow[:], in_=xrow[:], axis=mybir.AxisListType.X, op=ALU.max
            )
            for t in range(T):
                nc.vector.tensor_scalar(
                    out=zce[:, t, :], in0=xrow[:, t, :],
                    scalar1=mrow[:, t:t + 1], scalar2=None, op0=ALU.subtract,
                )
                nc.vector.tensor_scalar(
                    out=zbdc[:, t, :], in0=xrow[:, t, :],
                    scalar1=xy[:, t:t + 1], scalar2=None, op0=ALU.subtract,
                )
                nc.vector.tensor_scalar(
                    out=za[:, t, :], in0=xrow[:, t, :],
                    scalar1=xyea[:, t:t + 1], scalar2=None, op0=ALU.subtract,
                )
                nc.vector.tensor_scalar(
                    out=zb[:, t, :], in0=xrow[:, t, :],
                    scalar1=xyme[:, t:t + 1], scalar2=None, op0=ALU.subtract,
                )

            # ---- BEC hot loop ----
            for t in range(T):
                u = work.tile([P, NPAIR], F32, tag="u")
                for gi, (c0, nb) in enumerate(GROUPS):
                    pt = psum.tile([P, 4, CHUNK], F32, tag="dpsum")
                    for b in range(nb):
                        ci = c0 + b
                        q0 = ci * CHUNK
                        w = min(CHUNK, NMCOL - q0)
                        nc.tensor.matmul(
                            out=pt[:, b, 0:w],
                            lhsT=xthi[:, t * P:(t + 1) * P],
                            rhs=msb[:, q0:q0 + w],
                            start=True, stop=False,
                        )
                        nc.tensor.matmul(
                            out=pt[:, b, 0:w],
                            lhsT=xtlo[:, t * P:(t + 1) * P],
                            rhs=msb[:, q0:q0 + w],
                            start=False, stop=True,
                        )
                    # u = exp(-d) straight from PSUM
                    if gi < 2:
                        dst = u[:, c0 * CHUNK:(c0 + nb) * CHUNK].rearrange(
                            "p (a b) -> p a b", a=nb
                        )
                        nc.scalar.activation(
                            out=dst, in_=pt[:, :, :], func=AF.Exp, scale=-1.0
                        )
                    else:
                        nc.scalar.activation(
                            out=u[:, 8 * CHUNK:9 * CHUNK], in_=pt[:, 0, :],
                            func=AF.Exp, scale=-1.0,
                        )
                        nc.scalar.activation(
                            out=u[:, 9 * CHUNK:NPAIR],
                            in_=pt[:, 1, 0:NPAIR - 9 * CHUNK],
                            func=AF.Exp, scale=-1.0,
                        )
                        # signed pair-sum column (last col of the matmul)
                        nc.vector.tensor_copy(
                            out=sumd[:, t:t + 1],
                            in_=pt[:, 1, NPAIR - 9 * CHUNK:NMCOL - 9 * CHUNK],
                        )
                # ln(1+u), accumulate sum per partition
                nc.scalar.activation(
                    out=u[:], in_=u[:], func=AF.Ln, scale=1.0, bias=1.0,
                    accum_out=sumln[:, t:t + 1],
                )

            # ---- CE + BDC + A/B corrections (all exp/ln; same table set) ----
            nc.scalar.activation(out=zce[:], in_=zce[:], func=AF.Exp)
            nc.vector.tensor_reduce(
                out=se[:], in_=zce[:], axis=mybir.AxisListType.X, op=ALU.add
            )
            nc.scalar.activation(out=lnse[:], in_=se[:], func=AF.Ln)

            nc.scalar.activation(out=zbdc[:], in_=zbdc[:], func=AF.Exp)
            nc.scalar.activation(
                out=zbdc[:], in_=zbdc[:], func=AF.Ln, bias=1.0,
                accum_out=abt[:, 2:3],
            )
            nc.scalar.activation(out=za[:], in_=za[:], func=AF.Exp)
            nc.scalar.activation(
                out=za[:], in_=za[:], func=AF.Ln, bias=1.0,
                accum_out=abt[:, 0:1],
            )
            # b: u = exp(-(x - (xy - eps)))
            nc.scalar.activation(out=zb[:], in_=zb[:], func=AF.Exp, scale=-1.0)
            nc.scalar.activation(
                out=zb[:], in_=zb[:], func=AF.Ln, bias=1.0,
                accum_out=abt[:, 1:2],
            )

            # ---- write partials ----
            nc.sync.dma_start(out=sumd_d[:], in_=sumd[:])
            nc.sync.dma_start(out=sumln_d[:], in_=sumln[:])
            nc.sync.dma_start(out=abt_d[:], in_=abt[:])
            nc.sync.dma_start(out=lnse_d[:], in_=lnse[:])
            nc.sync.dma_start(out=mrow_d[:], in_=mrow[:])

    nc.compile()
    return nc


def _get_nc():
    if "nc" not in _cache:
        _cache["nc"] = _build_module()
    return _cache["nc"]


def _build_mmat():
    m = np.zeros((C, NMCOL), np.float32)
    q = np.arange(NPAIR)
    m[_PAIR_J, q] = 1.0
    m[_PAIR_K, q] = -1.0
    # last column: weights for sum over unordered pairs of (x_j - x_k)
    m[:, NPAIR] = (C - 1) - 2.0 * np.arange(C)
    return m.astype(ml_dtypes.bfloat16)


def _prep_core_inputs(Xs, xys, mmat_bf16):
    """Xs: [RPC, C] f32 shard; xys: [RPC] f32 target logits."""
    xrow = np.ascontiguousarray(
        Xs.reshape(T, P, C).transpose(1, 0, 2)
    )  # [P, T, C]
    xt = np.ascontiguousarray(Xs.T)  # [C, RPC] f32
    xthi = xt.astype(ml_dtypes.bfloat16)
    xtlo = (xt - xthi.astype(np.float32)).astype(ml_dtypes.bfloat16)
    xy = np.ascontiguousarray(xys.reshape(T, P).T)  # [P, T]
    return {
        "xrow": xrow,
        "xthi": xthi,
        "xtlo": xtlo,
        "mmat": mmat_bf16,
        "xy": xy,
        "xy_eps_a": (xy + np.float32(EPS)),
        "xy_meps": (xy - np.float32(EPS)),
    }


def _run(X, tgt, trace=False, tmpdir=None):
    nc = _get_nc()
    mmat_bf16 = _cache.get("mmat")
    if mmat_bf16 is None:
        mmat_bf16 = _cache["mmat"] = _build_mmat()

    xy_full = X[np.arange(N), tgt]
    in_maps = []
    for c in range(NCORES):
        sl = slice(c * RPC, (c + 1) * RPC)
        in_maps.append(_prep_core_inputs(X[sl], xy_full[sl], mmat_bf16))

    res = run_bass_kernel_spmd(
        nc, in_maps, core_ids=list(range(NCORES)), trace=trace, tmpdir=tmpdir
    )

    ls_eps = -math.log1p(math.exp(-EPS))
    log2 = math.log(2.0)

    ce_sum = 0.0
    t_sum = 0.0
    s_rest_sum = 0.0
    for c in range(NCORES):
        out = res.results[c]
        sumd = np.float64(out["sumd"]).sum()
        sumln = np.float64(out["sumln"]).sum()
        a_ln = np.float64(out["abt"][:, 0]).sum()
        b_ln = np.float64(out["abt"][:, 1]).sum()
        t_ln = np.float64(out["abt"][:, 2]).sum()
        lnse = np.float64(out["lnse"]).sum()
        mr = np.float64(out["mrow"]).sum()
        xy_c = np.float64(in_maps[c]["xy"]).sum()

        ce_sum += mr + lnse - xy_c
        t_sum += t_ln
        s_rest_sum += a_ln + b_ln - sumd - 2.0 * sumln + RPC * 101 * ls_eps

    loss_ce = ce_sum / N
    loss_bdc = (t_sum - N * log2) / ((C - 1) * N)
    loss_bec = -0.5 * s_rest_sum / ((C - 1) * (C - 2) * N)
    loss = loss_ce + loss_bdc + loss_bec
    outs = tuple(
        np.float32(v) for v in (loss, loss_ce, loss_bdc, loss_bec)
    )
    return outs, res


def kernel(inputs, targets):
    X = np.ascontiguousarray(np.asarray(inputs, dtype=np.float32))
    tgt = np.asarray(targets).astype(np.int64)
    assert X.shape == (N, C), X.shape
    outs, _ = _run(X, tgt, trace=False)
    return outs


# revision 11
# speedup vs baseline: 1.2393x; 1.2393x over previous
"""CPCLoss (CE + BDC + BEC) Trainium2 kernel.

Strategy (data-parallel over N across 8 NeuronCores, 1024 rows/core):
  Per row, the BEC term needs sum over all ordered pairs (j,k) of
  logsigmoid(x_j - x_k (+eps)).  Using the identity
      softplus(d) + softplus(-d) = d + 2*ln(1 + exp(-d))
  only the 4950 unordered pairs are evaluated; the signed-pair sum
  sum_q d_q is a fixed linear functional of the row, computed as one extra
  matmul column with weights (C-1-2i).  On device:
    - TensorE computes all pair differences d = x_j - x_k via a constant
      {+1,-1} difference matrix (bf16 hi/lo split of x for fp32-grade
      precision), accumulated in PSUM.
    - ScalarE reads PSUM directly: exp(-d), then ln(1 + u) over SBUF (both
      functions live in the single 'natural_log_exp_and_others' activation
      table set -> one table load for the whole kernel), accumulating
      sum ln(1+e^-d) per partition via accum_out.
  CE (logsumexp) and BDC plus the two BEC "target row/col" corrections are
  tiny (O(N*C)) and use the same exp/ln tables.
  Per-core partial sums land in small [128, k] tensors; the host combines
  them in float64.
"""

import math
import os
import sys

sys.path.insert(0, "/opt/trn_rl_repo")

import numpy as np
import ml_dtypes

import concourse.bacc as bacc
import concourse.tile as tile
from concourse import mybir
from concourse.bass_utils import run_bass_kernel_spmd

F32 = mybir.dt.float32
BF16 = mybir.dt.bfloat16
AF = mybir.ActivationFunctionType
ALU = mybir.AluOpType

N, C = 8192, 100
NCORES = 8
RPC = N // NCORES          # rows per core = 1024
P = 128                    # partitions
T = RPC // P               # row-tiles per core = 8
EPS = 1e-7
NPAIR = (C * (C - 1)) // 2  # 4950
NMCOL = NPAIR + 1           # extra col: sum_q d_q weights
CHUNK = 512
NCHUNK = (NMCOL + CHUNK - 1) // CHUNK   # 10 (last = 343)
# psum groups: chunks [0..3], [4..7], [8..9]
GROUPS = [(0, 4), (4, 4), (8, 2)]

_PAIR_J, _PAIR_K = np.triu_indices(C, 1)

_cache = {}


def _patch_act_tables():
    """Steer the activation-table allocator so Exp and Ln both resolve to the
    combined 'natural_log_exp_and_others' set (one ACT_TABLE_LOAD total,
    ~1.3us) instead of bouncing between 'exp_and_others' and 'natural_log'
    (a 1.3us reload on every switch).  Set order/length is preserved so
    act_func_set_id still indexes the real act_info.json."""
    if _cache.get("act_patched"):
        return
    from concourse.hw_specs import get_activation_tables as _real

    def _patched(arch):
        tabs = {k: set(v) for k, v in _real(arch).items()}
        for name, fns in tabs.items():
            if name != "natural_log_exp_and_others":
                fns.discard(AF.Exp)
                fns.discard(AF.Ln)
        return tabs

    bacc.get_activation_tables = _patched
    _cache["act_patched"] = True


def _build_module():
    _patch_act_tables()
    nc = bacc.Bacc("TRN2", target_bir_lowering=False, debug=False)

    xrow_d = nc.dram_tensor("xrow", [P, T, C], F32, kind="ExternalInput")
    xthi_d = nc.dram_tensor("xthi", [C, RPC], BF16, kind="ExternalInput")
    xtlo_d = nc.dram_tensor("xtlo", [C, RPC], BF16, kind="ExternalInput")
    mmat_d = nc.dram_tensor("mmat", [C, NMCOL], BF16, kind="ExternalInput")
    xy_d = nc.dram_tensor("xy", [P, T], F32, kind="ExternalInput")
    xyea_d = nc.dram_tensor("xy_eps_a", [P, T], F32, kind="ExternalInput")
    xyme_d = nc.dram_tensor("xy_meps", [P, T], F32, kind="ExternalInput")

    sumd_d = nc.dram_tensor("sumd", [P, T], F32, kind="ExternalOutput")
    sumln_d = nc.dram_tensor("sumln", [P, T], F32, kind="ExternalOutput")
    abt_d = nc.dram_tensor("abt", [P, 3], F32, kind="ExternalOutput")
    lnse_d = nc.dram_tensor("lnse", [P, T], F32, kind="ExternalOutput")
    mrow_d = nc.dram_tensor("mrow", [P, T], F32, kind="ExternalOutput")

    with tile.TileContext(nc) as tc:
        with (
            tc.tile_pool(name="consts", bufs=1) as consts,
            tc.tile_pool(name="work", bufs=2) as work,
            tc.tile_pool(name="psum", bufs=2, space="PSUM") as psum,
        ):
            # ---- load inputs ----
            xrow = consts.tile([P, T, C], F32)
            nc.sync.dma_start(out=xrow[:], in_=xrow_d[:])
            xthi = consts.tile([C, RPC], BF16)
            nc.sync.dma_start(out=xthi[:], in_=xthi_d[:])
            xtlo = consts.tile([C, RPC], BF16)
            nc.sync.dma_start(out=xtlo[:], in_=xtlo_d[:])
            xy = consts.tile([P, T], F32)
            nc.sync.dma_start(out=xy[:], in_=xy_d[:])
            xyea = consts.tile([P, T], F32)
            nc.sync.dma_start(out=xyea[:], in_=xyea_d[:])
            xyme = consts.tile([P, T], F32)
            nc.sync.dma_start(out=xyme[:], in_=xyme_d[:])
            # difference matrix, chunked DMA so early matmuls start sooner
            msb = consts.tile([C, NMCOL], BF16)
            for ci in range(NCHUNK):
                q0 = ci * CHUNK
                w = min(CHUNK, NMCOL - q0)
                nc.sync.dma_start(out=msb[:, q0:q0 + w], in_=mmat_d[:, q0:q0 + w])

            # ---- accumulators ----
            sumd = consts.tile([P, T], F32)
            sumln = consts.tile([P, T], F32)
            abt = consts.tile([P, 3], F32)
            lnse = consts.tile([P, T], F32)
            mrow = consts.tile([P, T], F32)
            se = consts.tile([P, T], F32)
            zce = consts.tile([P, T, C], F32)
            zbdc = consts.tile([P, T, C], F32)
            za = consts.tile([P, T, C], F32)
            zb = consts.tile([P, T, C], F32)

            # ---- CE prep: row max then x - m (per tile; scalar1 is per-partition) ----
            nc.vector.tensor_reduce(
                out=mrow[:], in_=xrow[:], axis=mybir.AxisListType.X, op=ALU.max
            )
            for t in range(T):
                nc.vector.tensor_scalar(
                    out=zce[:, t, :], in0=xrow[:, t, :],
                    scalar1=mrow[:, t:t + 1], scalar2=None, op0=ALU.subtract,
                )
                nc.vector.tensor_scalar(
                    out=zbdc[:, t, :], in0=xrow[:, t, :],
                    scalar1=xy[:, t:t + 1], scalar2=None, op0=ALU.subtract,
                )
                nc.vector.tensor_scalar(
                    out=za[:, t, :], in0=xrow[:, t, :],
                    scalar1=xyea[:, t:t + 1], scalar2=None, op0=ALU.subtract,
                )
                nc.vector.tensor_scalar(
                    out=zb[:, t, :], in0=xrow[:, t, :],
                    scalar1=xyme[:, t:t + 1], scalar2=None, op0=ALU.subtract,
                )

            # ---- BEC hot loop ----
            for t in range(T):
                u = work.tile([P, NPAIR], F32, tag="u")
                for gi, (c0, nb) in enumerate(GROUPS):
                    pt = psum.tile([P, 4, CHUNK], F32, tag="dpsum")
                    for b in range(nb):
                        ci = c0 + b
                        q0 = ci * CHUNK
                        w = min(CHUNK, NMCOL - q0)
                        nc.tensor.matmul(
                            out=pt[:, b, 0:w],
                            lhsT=xthi[:, t * P:(t + 1) * P],
                            rhs=msb[:, q0:q0 + w],
                            start=True, stop=False,
                        )
                        nc.tensor.matmul(
                            out=pt[:, b, 0:w],
                            lhsT=xtlo[:, t * P:(t + 1) * P],
                            rhs=msb[:, q0:q0 + w],
                            start=False, stop=True,
                        )
                    # u = exp(-d) straight from PSUM
                    if gi < 2:
                        dst = u[:, c0 * CHUNK:(c0 + nb) * CHUNK].rearrange(
                            "p (a b) -> p a b", a=nb
                        )
                        nc.scalar.activation(
                            out=dst, in_=pt[:, :, :], func=AF.Exp, scale=-1.0
                        )
                    else:
                        nc.scalar.activation(
                            out=u[:, 8 * CHUNK:9 * CHUNK], in_=pt[:, 0, :],
                            func=AF.Exp, scale=-1.0,
                        )
                        nc.scalar.activation(
                            out=u[:, 9 * CHUNK:NPAIR],
                            in_=pt[:, 1, 0:NPAIR - 9 * CHUNK],
                            func=AF.Exp, scale=-1.0,
                        )
                        # signed pair-sum column (last col of the matmul)
                        nc.vector.tensor_copy(
                            out=sumd[:, t:t + 1],
                            in_=pt[:, 1, NPAIR - 9 * CHUNK:NMCOL - 9 * CHUNK],
                        )
                # ln(1+u), accumulate sum per partition
                nc.scalar.activation(
                    out=u[:], in_=u[:], func=AF.Ln, scale=1.0, bias=1.0,
                    accum_out=sumln[:, t:t + 1],
                )

            # ---- CE + BDC + A/B corrections (all exp/ln; same table set) ----
            nc.scalar.activation(out=zce[:], in_=zce[:], func=AF.Exp)
            nc.vector.tensor_reduce(
                out=se[:], in_=zce[:], axis=mybir.AxisListType.X, op=ALU.add
            )
            nc.scalar.activation(out=lnse[:], in_=se[:], func=AF.Ln)

            nc.scalar.activation(out=zbdc[:], in_=zbdc[:], func=AF.Exp)
            nc.scalar.activation(
                out=zbdc[:], in_=zbdc[:], func=AF.Ln, bias=1.0,
                accum_out=abt[:, 2:3],
            )
            nc.scalar.activation(out=za[:], in_=za[:], func=AF.Exp)
            nc.scalar.activation(
                out=za[:], in_=za[:], func=AF.Ln, bias=1.0,
                accum_out=abt[:, 0:1],
            )
            # b: u = exp(-(x - (xy - eps)))
            nc.scalar.activation(out=zb[:], in_=zb[:], func=AF.Exp, scale=-1.0)
            nc.scalar.activation(
                out=zb[:], in_=zb[:], func=AF.Ln, bias=1.0,
                accum_out=abt[:, 1:2],
            )

            # ---- write partials ----
            nc.sync.dma_start(out=sumd_d[:], in_=sumd[:])
            nc.sync.dma_start(out=sumln_d[:], in_=sumln[:])
            nc.sync.dma_start(out=abt_d[:], in_=abt[:])
            nc.sync.dma_start(out=lnse_d[:], in_=lnse[:])
            nc.sync.dma_start(out=mrow_d[:], in_=mrow[:])

    nc.compile()
    return nc


def _get_nc():
    if "nc" not in _cache:
        _cache["nc"] = _build_module()
    return _cache["nc"]


def _build_mmat():
    m = np.zeros((C, NMCOL), np.float32)
    q = np.arange(NPAIR)
    m[_PAIR_J, q] = 1.0
    m[_PAIR_K, q] = -1.0
    # last column: weights for sum over unordered pairs of (x_j - x_k)
    m[:, NPAIR] = (C - 1) - 2.0 * np.arange(C)
    return m.astype(ml_dtypes.bfloat16)


def _prep_core_inputs(Xs, xys, mmat_bf16):
    """Xs: [RPC, C] f32 shard; xys: [RPC] f32 target logits."""
    xrow = np.ascontiguousarray(
        Xs.reshape(T, P, C).transpose(1, 0, 2)
    )  # [P, T, C]
    xt = np.ascontiguousarray(Xs.T)  # [C, RPC] f32
    xthi = xt.astype(ml_dtypes.bfloat16)
    xtlo = (xt - xthi.astype(np.float32)).astype(ml_dtypes.bfloat16)
    xy = np.ascontiguousarray(xys.reshape(T, P).T)  # [P, T]
    return {
        "xrow": xrow,
        "xthi": xthi,
        "xtlo": xtlo,
        "mmat": mmat_bf16,
        "xy": xy,
        "xy_eps_a": (xy + np.float32(EPS)),
        "xy_meps": (xy - np.float32(EPS)),
    }


def _run(X, tgt, trace=False, tmpdir=None):
    nc = _get_nc()
    mmat_bf16 = _cache.get("mmat")
    if mmat_bf16 is None:
        mmat_bf16 = _cache["mmat"] = _build_mmat()

    xy_full = X[np.arange(N), tgt]
    in_maps = []
    for c in range(NCORES):
        sl = slice(c * RPC, (c + 1) * RPC)
        in_maps.append(_prep_core_inputs(X[sl], xy_full[sl], mmat_bf16))

    res = run_bass_kernel_spmd(
        nc, in_maps, core_ids=list(range(NCORES)), trace=trace, tmpdir=tmpdir
    )

    ls_eps = -math.log1p(math.exp(-EPS))
    log2 = math.log(2.0)

    ce_sum = 0.0
    t_sum = 0.0
    s_rest_sum = 0.0
    for c in range(NCORES):
        out = res.results[c]
        sumd = np.float64(out["sumd"]).sum()
        sumln = np.float64(out["sumln"]).sum()
        a_ln = np.float64(out["abt"][:, 0]).sum()
        b_ln = np.float64(out["abt"][:, 1]).sum()
        t_ln = np.float64(out["abt"][:, 2]).sum()
        lnse = np.float64(out["lnse"]).sum()
        mr = np.float64(out["mrow"]).sum()
        xy_c = np.float64(in_maps[c]["xy"]).sum()

        ce_sum += mr + lnse - xy_c
        t_sum += t_ln
        s_rest_sum += a_ln + b_ln - sumd - 2.0 * sumln + RPC * 101 * ls_eps

    loss_ce = ce_sum / N
    loss_bdc = (t_sum - N * log2) / ((C - 1) * N)
    loss_bec = -0.5 * s_rest_sum / ((C - 1) * (C - 2) * N)
    loss = loss_ce + loss_bdc + loss_bec
    outs = tuple(
        np.float32(v) for v in (loss, loss_ce, loss_bdc, loss_bec)
    )
    return outs, res


def kernel(inputs, targets):
    X = np.ascontiguousarray(np.asarray(inputs, dtype=np.float32))
    tgt = np.asarray(targets).astype(np.int64)
    assert X.shape == (N, C), X.shape
    outs, _ = _run(X, tgt, trace=False)
    return outs


# revision 12
# speedup vs baseline: 1.2394x; 1.0001x over previous
"""CPCLoss (CE + BDC + BEC) Trainium2 kernel.

Data-parallel over N across 8 NeuronCores (1024 rows/core).  Per row, BEC
needs sum over ordered class pairs (j,k) of logsigmoid(x_j - x_k + eps).
With sp(z) = ln(1+e^z):   sp(d) + sp(-d) = d + 2*sp(-d)
so only the 4950 unordered pair diffs are evaluated nonlinearly; the signed
linear parts (sum of pair diffs, row sums) are exact linear functionals the
host computes directly in float64.

On device, per 128-row tile:
  - TensorE: all pair diffs d = x_j - x_k as one matmul with a constant
    {+1,-1} difference matrix (bf16 hi/lo split of x keeps ~2^-18 accuracy,
    with exact fp32 PSUM accumulation).
  - ScalarE: u = exp(-d) straight from PSUM, then ln(1+u) over SBUF with
    per-partition accumulation (accum_out).  Both functions live in one
    activation table set ('natural_log_exp_and_others', selection steered
    via _patch_act_tables) -> a single ACT_TABLE_LOAD for the whole kernel.
  - CE logsumexp and the BEC target-row correction a_ln reuse exp/ln.
BDC and the second BEC correction differ from a_ln only by linear terms
(and O(eps) wiggle far below fp32 noise), so the host derives them.
"""

import math
import sys

sys.path.insert(0, "/opt/trn_rl_repo")

import numpy as np
import ml_dtypes

import concourse.bacc as bacc
import concourse.tile as tile
from concourse import mybir
from concourse.bass_utils import run_bass_kernel_spmd

F32 = mybir.dt.float32
BF16 = mybir.dt.bfloat16
AF = mybir.ActivationFunctionType
ALU = mybir.AluOpType

N, C = 8192, 100
NCORES = 8
RPC = N // NCORES          # rows per core = 1024
P = 128                    # partitions
T = RPC // P               # row-tiles per core = 8
EPS = 1e-7
NPAIR = (C * (C - 1)) // 2  # 4950
CHUNK = 512
NCHUNK = (NPAIR + CHUNK - 1) // CHUNK   # 10 (last = 342)
# psum groups: chunks [0..3], [4..7], [8..9]
GROUPS = [(0, 4), (4, 4), (8, 2)]

_PAIR_J, _PAIR_K = np.triu_indices(C, 1)

_cache = {}


def _patch_act_tables():
    """Steer the activation-table allocator so Exp and Ln both resolve to the
    combined 'natural_log_exp_and_others' set (one ACT_TABLE_LOAD total,
    ~1.3us) instead of bouncing between 'exp_and_others' and 'natural_log'
    (a 1.3us reload on every switch).  Set order/length is preserved so
    act_func_set_id still indexes the real act_info.json."""
    if _cache.get("act_patched"):
        return
    from concourse.hw_specs import get_activation_tables as _real

    def _patched(arch):
        tabs = {k: set(v) for k, v in _real(arch).items()}
        for name, fns in tabs.items():
            if name != "natural_log_exp_and_others":
                fns.discard(AF.Exp)
                fns.discard(AF.Ln)
        return tabs

    bacc.get_activation_tables = _patched
    _cache["act_patched"] = True


def _build_module():
    _patch_act_tables()
    nc = bacc.Bacc("TRN2", target_bir_lowering=False, debug=False)

    xthi_d = nc.dram_tensor("xthi", [C, RPC], BF16, kind="ExternalInput")
    xtlo_d = nc.dram_tensor("xtlo", [C, RPC], BF16, kind="ExternalInput")
    mmat_d = nc.dram_tensor("mmat", [C, NPAIR], BF16, kind="ExternalInput")
    xrow_d = nc.dram_tensor("xrow", [P, T, C], F32, kind="ExternalInput")
    xyea_d = nc.dram_tensor("xy_eps_a", [P, T], F32, kind="ExternalInput")

    # parts: 0:8 sumln | 8:16 lnse | 16:24 mrow | 24 a_ln
    parts_d = nc.dram_tensor("parts", [P, 25], F32, kind="ExternalOutput")

    with tile.TileContext(nc) as tc:
        with (
            tc.tile_pool(name="consts", bufs=1) as consts,
            tc.tile_pool(name="work", bufs=2) as work,
            tc.tile_pool(name="psum", bufs=2, space="PSUM") as psum,
        ):
            # ---- load inputs (critical-path first) ----
            xthi = consts.tile([C, RPC], BF16)
            nc.sync.dma_start(out=xthi[:], in_=xthi_d[:])
            xtlo = consts.tile([C, RPC], BF16)
            nc.sync.dma_start(out=xtlo[:], in_=xtlo_d[:])
            msb = consts.tile([C, NPAIR], BF16)
            for ci in range(NCHUNK):
                q0 = ci * CHUNK
                w = min(CHUNK, NPAIR - q0)
                nc.sync.dma_start(out=msb[:, q0:q0 + w], in_=mmat_d[:, q0:q0 + w])
            xrow = consts.tile([P, T, C], F32)
            nc.sync.dma_start(out=xrow[:], in_=xrow_d[:])
            xyea = consts.tile([P, T], F32)
            nc.sync.dma_start(out=xyea[:], in_=xyea_d[:])

            # ---- accumulators / small work ----
            parts = consts.tile([P, 25], F32)
            sumln = parts[:, 0:8]
            lnse = parts[:, 8:16]
            mrow = parts[:, 16:24]
            aln = parts[:, 24:25]
            se = consts.tile([P, T], F32)
            zce = consts.tile([P, T, C], F32)
            za = consts.tile([P, T, C], F32)

            # ---- CE prep + a_ln prep on DVE ----
            nc.vector.tensor_reduce(
                out=mrow, in_=xrow[:], axis=mybir.AxisListType.X, op=ALU.max
            )
            for t in range(T):
                nc.vector.tensor_scalar(
                    out=zce[:, t, :], in0=xrow[:, t, :],
                    scalar1=mrow[:, t:t + 1], scalar2=None, op0=ALU.subtract,
                )
                nc.vector.tensor_scalar(
                    out=za[:, t, :], in0=xrow[:, t, :],
                    scalar1=xyea[:, t:t + 1], scalar2=None, op0=ALU.subtract,
                )

            # ---- BEC hot loop ----
            for t in range(T):
                u = work.tile([P, NPAIR], F32, tag="u")
                for gi, (c0, nb) in enumerate(GROUPS):
                    pt = psum.tile([P, 4, CHUNK], F32, tag="dpsum")
                    for b in range(nb):
                        ci = c0 + b
                        q0 = ci * CHUNK
                        w = min(CHUNK, NPAIR - q0)
                        nc.tensor.matmul(
                            out=pt[:, b, 0:w],
                            lhsT=xthi[:, t * P:(t + 1) * P],
                            rhs=msb[:, q0:q0 + w],
                            start=True, stop=False,
                        )
                        nc.tensor.matmul(
                            out=pt[:, b, 0:w],
                            lhsT=xtlo[:, t * P:(t + 1) * P],
                            rhs=msb[:, q0:q0 + w],
                            start=False, stop=True,
                        )
                    # u = exp(-d) straight from PSUM
                    if gi < 2:
                        dst = u[:, c0 * CHUNK:(c0 + nb) * CHUNK].rearrange(
                            "p (a b) -> p a b", a=nb
                        )
                        nc.scalar.activation(
                            out=dst, in_=pt[:, :, :], func=AF.Exp, scale=-1.0
                        )
                    else:
                        nc.scalar.activation(
                            out=u[:, 8 * CHUNK:9 * CHUNK], in_=pt[:, 0, :],
                            func=AF.Exp, scale=-1.0,
                        )
                        nc.scalar.activation(
                            out=u[:, 9 * CHUNK:NPAIR],
                            in_=pt[:, 1, 0:NPAIR - 9 * CHUNK],
                            func=AF.Exp, scale=-1.0,
                        )
                # ln(1+u), accumulate sum per partition
                nc.scalar.activation(
                    out=u[:], in_=u[:], func=AF.Ln, scale=1.0, bias=1.0,
                    accum_out=sumln[:, t:t + 1],
                )

            # ---- CE + a_ln (same exp/ln table set) ----
            nc.scalar.activation(out=zce[:], in_=zce[:], func=AF.Exp)
            nc.vector.tensor_reduce(
                out=se[:], in_=zce[:], axis=mybir.AxisListType.X, op=ALU.add
            )
            nc.scalar.activation(out=lnse, in_=se[:], func=AF.Ln)

            nc.scalar.activation(out=za[:], in_=za[:], func=AF.Exp)
            nc.scalar.activation(
                out=za[:], in_=za[:], func=AF.Ln, bias=1.0, accum_out=aln
            )

            # ---- write partials ----
            nc.sync.dma_start(out=parts_d[:], in_=parts[:])

    nc.compile()
    return nc


def _get_nc():
    if "nc" not in _cache:
        _cache["nc"] = _build_module()
    return _cache["nc"]


def _build_mmat():
    m = np.zeros((C, NPAIR), np.float32)
    q = np.arange(NPAIR)
    m[_PAIR_J, q] = 1.0
    m[_PAIR_K, q] = -1.0
    return m.astype(ml_dtypes.bfloat16)


def _prep_core_inputs(Xs, xys, mmat_bf16):
    """Xs: [RPC, C] f32 shard; xys: [RPC] f32 target logits."""
    xrow = np.ascontiguousarray(
        Xs.reshape(T, P, C).transpose(1, 0, 2)
    )  # [P, T, C]
    xt = np.ascontiguousarray(Xs.T)  # [C, RPC] f32
    xthi = xt.astype(ml_dtypes.bfloat16)
    xtlo = (xt - xthi.astype(np.float32)).astype(ml_dtypes.bfloat16)
    xy = np.ascontiguousarray(xys.reshape(T, P).T)  # [P, T]
    return {
        "xrow": xrow,
        "xthi": xthi,
        "xtlo": xtlo,
        "mmat": mmat_bf16,
        "xy_eps_a": (xy + np.float32(EPS)),
    }


def _run(X, tgt, trace=False, tmpdir=None):
    nc = _get_nc()
    mmat_bf16 = _cache.get("mmat")
    if mmat_bf16 is None:
        mmat_bf16 = _cache["mmat"] = _build_mmat()

    xy_full = X[np.arange(N), tgt]
    in_maps = []
    for c in range(NCORES):
        sl = slice(c * RPC, (c + 1) * RPC)
        in_maps.append(_prep_core_inputs(X[sl], xy_full[sl], mmat_bf16))

    res = run_bass_kernel_spmd(
        nc, in_maps, core_ids=list(range(NCORES)), trace=trace, tmpdir=tmpdir
    )

    # ---- host-side exact linear functionals (float64) ----
    X64 = np.float64(X)
    xy64 = np.float64(xy_full)
    wvec = (C - 1) - 2.0 * np.arange(C, dtype=np.float64)
    sumd = (X64 @ wvec).sum()          # sum over rows of sum_{j<k}(x_j - x_k)
    xsum = X64.sum()
    xysum = xy64.sum()

    ls_eps = -math.log1p(math.exp(-EPS))
    log2 = math.log(2.0)

    sumln_tot = 0.0
    a_tot = 0.0
    mlnse_tot = 0.0
    for c in range(NCORES):
        parts = np.float64(res.results[c]["parts"])
        sumln_tot += parts[:, 0:8].sum()
        mlnse_tot += parts[:, 8:24].sum()   # lnse + mrow together
        a_tot += parts[:, 24].sum()

    t_sum = a_tot
    b_sum = a_tot - (xsum - C * xysum - N * C * EPS)

    ce_sum = mlnse_tot - xysum
    s_rest = a_tot + b_sum - sumd - 2.0 * sumln_tot + N * 101 * ls_eps

    loss_ce = ce_sum / N
    loss_bdc = (t_sum - N * log2) / ((C - 1) * N)
    loss_bec = -0.5 * s_rest / ((C - 1) * (C - 2) * N)
    loss = loss_ce + loss_bdc + loss_bec
    outs = tuple(
        np.float32(v) for v in (loss, loss_ce, loss_bdc, loss_bec)
    )
    return outs, res


def kernel(inputs, targets):
    X = np.ascontiguousarray(np.asarray(inputs, dtype=np.float32))
    tgt = np.asarray(targets).astype(np.int64)
    assert X.shape == (N, C), X.shape
    outs, _ = _run(X, tgt, trace=False)
    return outs


# revision 16
# speedup vs baseline: 1.2726x; 1.0267x over previous
"""CPCLoss (CE + BDC + BEC) Trainium2 kernel.

Data-parallel over N across 8 NeuronCores (1024 rows/core).  Per row, BEC
needs sum over ordered class pairs (j,k) of logsigmoid(x_j - x_k + eps).
With sp(z) = ln(1+e^z):   sp(d) + sp(-d) = d + 2*sp(-d)
so only the 4950 unordered pair diffs are evaluated nonlinearly; the signed
linear parts (sum of pair diffs, row sums) are exact linear functionals the
host computes directly in float64.

On device, per 128-row tile:
  - TensorE: all pair diffs d = x_j - x_k as one matmul with a constant
    {+1,-1} difference matrix (bf16 hi/lo split of x keeps ~2^-18 accuracy,
    with exact fp32 PSUM accumulation).
  - ScalarE: u = exp(-d) straight from PSUM, then ln(1+u) over SBUF with
    per-partition accumulation (accum_out).  Both functions live in one
    activation table set ('natural_log_exp_and_others', selection steered
    via _patch_act_tables) -> a single ACT_TABLE_LOAD for the whole kernel.
  - CE logsumexp and the BEC target-row correction a_ln reuse exp/ln.
BDC and the second BEC correction differ from a_ln only by linear terms
(and O(eps) wiggle far below fp32 noise), so the host derives them.
"""

import math
import sys

sys.path.insert(0, "/opt/trn_rl_repo")

import numpy as np
import ml_dtypes

import concourse.bacc as bacc
import concourse.tile as tile
from concourse import mybir
from concourse.bass_utils import run_bass_kernel_spmd

F32 = mybir.dt.float32
BF16 = mybir.dt.bfloat16
AF = mybir.ActivationFunctionType
ALU = mybir.AluOpType

N, C = 8192, 100
NCORES = 8
RPC = N // NCORES          # rows per core = 1024
P = 128                    # partitions
T = RPC // P               # row-tiles per core = 8
EPS = 1e-7
NPAIR = (C * (C - 1)) // 2  # 4950
CHUNK = 495                 # 10 uniform chunks; 2 chunks per PSUM group
NCHUNK = NPAIR // CHUNK     # 10
NGRP = 5                    # groups of 2 banks -> 4 psum slots of 2 banks

_PAIR_J, _PAIR_K = np.triu_indices(C, 1)

_cache = {}


def _patch_act_tables():
    """Steer the activation-table allocator so Exp and Ln both resolve to the
    combined 'natural_log_exp_and_others' set (one ACT_TABLE_LOAD total,
    ~1.3us) instead of bouncing between 'exp_and_others' and 'natural_log'
    (a 1.3us reload on every switch).  Set order/length is preserved so
    act_func_set_id still indexes the real act_info.json."""
    if _cache.get("act_patched"):
        return
    from concourse.hw_specs import get_activation_tables as _real

    def _patched(arch):
        tabs = {k: set(v) for k, v in _real(arch).items()}
        for name, fns in tabs.items():
            if name != "natural_log_exp_and_others":
                fns.discard(AF.Exp)
                fns.discard(AF.Ln)
        return tabs

    bacc.get_activation_tables = _patched
    _cache["act_patched"] = True


def _build_module():
    _patch_act_tables()
    nc = bacc.Bacc("TRN2", target_bir_lowering=False, debug=False)

    xthi_d = nc.dram_tensor("xthi", [C, RPC], BF16, kind="ExternalInput")
    xtlo_d = nc.dram_tensor("xtlo", [C, RPC], BF16, kind="ExternalInput")
    mmat_d = nc.dram_tensor("mmat", [C, NPAIR], BF16, kind="ExternalInput")
    xrow_d = nc.dram_tensor("xrow", [P, T, C], F32, kind="ExternalInput")
    xyea_d = nc.dram_tensor("xy_eps_a", [P, T], F32, kind="ExternalInput")

    # parts: 0:8 sumln | 8:16 lnse | 16:24 mrow | 24 a_ln
    parts_d = nc.dram_tensor("parts", [P, 25], F32, kind="ExternalOutput")

    with tile.TileContext(nc) as tc:
        with (
            tc.tile_pool(name="consts", bufs=1) as consts,
            tc.tile_pool(name="work", bufs=2) as work,
            tc.tile_pool(name="psum", bufs=4, space="PSUM") as psum,
        ):
            # ---- load inputs (critical-path first) ----
            xthi = consts.tile([C, RPC], BF16)
            nc.sync.dma_start(out=xthi[:], in_=xthi_d[:])
            xtlo = consts.tile([C, RPC], BF16)
            nc.sync.dma_start(out=xtlo[:], in_=xtlo_d[:])
            msb = consts.tile([C, NPAIR], BF16)
            for ci in range(NCHUNK):
                q0 = ci * CHUNK
                nc.sync.dma_start(
                    out=msb[:, q0:q0 + CHUNK], in_=mmat_d[:, q0:q0 + CHUNK]
                )
            xrow = consts.tile([P, T, C], F32)
            nc.sync.dma_start(out=xrow[:], in_=xrow_d[:])
            xyea = consts.tile([P, T], F32)
            nc.sync.dma_start(out=xyea[:], in_=xyea_d[:])

            # ---- accumulators / small work ----
            parts = consts.tile([P, 25], F32)
            sumln = parts[:, 0:8]
            lnse = parts[:, 8:16]
            mrow = parts[:, 16:24]
            aln = parts[:, 24:25]
            se = consts.tile([P, T], F32)
            zce = consts.tile([P, T, C], F32)
            za = consts.tile([P, T, C], F32)

            # ---- CE prep + a_ln prep on DVE ----
            nc.vector.tensor_reduce(
                out=mrow, in_=xrow[:], axis=mybir.AxisListType.X, op=ALU.max
            )
            for t in range(T):
                nc.vector.tensor_scalar(
                    out=zce[:, t, :], in0=xrow[:, t, :],
                    scalar1=mrow[:, t:t + 1], scalar2=None, op0=ALU.subtract,
                )
                nc.vector.tensor_scalar(
                    out=za[:, t, :], in0=xrow[:, t, :],
                    scalar1=xyea[:, t:t + 1], scalar2=None, op0=ALU.subtract,
                )

            # ---- CE + a_ln first (fills ACT while first matmuls ramp) ----
            nc.scalar.activation(out=zce[:], in_=zce[:], func=AF.Exp)
            nc.vector.tensor_reduce(
                out=se[:], in_=zce[:], axis=mybir.AxisListType.X, op=ALU.add
            )
            nc.scalar.activation(out=za[:], in_=za[:], func=AF.Exp)
            nc.scalar.activation(
                out=za[:], in_=za[:], func=AF.Ln, bias=1.0, accum_out=aln
            )
            nc.scalar.activation(out=lnse, in_=se[:], func=AF.Ln)

            # ---- BEC hot loop ----
            for t in range(T):
                u = work.tile([P, NPAIR], F32, tag="u")
                for g in range(NGRP):
                    pt = psum.tile([P, 2, 512], F32, tag="dpsum")
                    for b in range(2):
                        q0 = (g * 2 + b) * CHUNK
                        nc.tensor.matmul(
                            out=pt[:, b, 0:CHUNK],
                            lhsT=xthi[:, t * P:(t + 1) * P],
                            rhs=msb[:, q0:q0 + CHUNK],
                            start=True, stop=False,
                        )
                    for b in range(2):
                        q0 = (g * 2 + b) * CHUNK
                        nc.tensor.matmul(
                            out=pt[:, b, 0:CHUNK],
                            lhsT=xtlo[:, t * P:(t + 1) * P],
                            rhs=msb[:, q0:q0 + CHUNK],
                            start=False, stop=True,
                        )
                    # u = exp(-d) straight from PSUM
                    dst = u[:, g * 2 * CHUNK:(g + 1) * 2 * CHUNK].rearrange(
                        "p (a b) -> p a b", a=2
                    )
                    nc.scalar.activation(
                        out=dst, in_=pt[:, :, 0:CHUNK], func=AF.Exp, scale=-1.0
                    )
                # ln(1+u), accumulate sum per partition
                nc.scalar.activation(
                    out=u[:], in_=u[:], func=AF.Ln, scale=1.0, bias=1.0,
                    accum_out=sumln[:, t:t + 1],
                )

            # ---- write partials ----
            nc.sync.dma_start(out=parts_d[:], in_=parts[:])

    nc.compile()
    return nc


def _get_nc():
    if "nc" not in _cache:
        _cache["nc"] = _build_module()
    return _cache["nc"]


def _build_mmat():
    m = np.zeros((C, NPAIR), np.float32)
    q = np.arange(NPAIR)
    m[_PAIR_J, q] = 1.0
    m[_PAIR_K, q] = -1.0
    return m.astype(ml_dtypes.bfloat16)


def _prep_core_inputs(Xs, xys, mmat_bf16):
    """Xs: [RPC, C] f32 shard; xys: [RPC] f32 target logits."""
    xrow = np.ascontiguousarray(
        Xs.reshape(T, P, C).transpose(1, 0, 2)
    )  # [P, T, C]
    xt = np.ascontiguousarray(Xs.T)  # [C, RPC] f32
    xthi = xt.astype(ml_dtypes.bfloat16)
    xtlo = (xt - xthi.astype(np.float32)).astype(ml_dtypes.bfloat16)
    xy = np.ascontiguousarray(xys.reshape(T, P).T)  # [P, T]
    return {
        "xrow": xrow,
        "xthi": xthi,
        "xtlo": xtlo,
        "mmat": mmat_bf16,
        "xy_eps_a": (xy + np.float32(EPS)),
    }


def _run(X, tgt, trace=False, tmpdir=None):
    nc = _get_nc()
    mmat_bf16 = _cache.get("mmat")
    if mmat_bf16 is None:
        mmat_bf16 = _cache["mmat"] = _build_mmat()

    xy_full = X[np.arange(N), tgt]
    in_maps = []
    for c in range(NCORES):
        sl = slice(c * RPC, (c + 1) * RPC)
        in_maps.append(_prep_core_inputs(X[sl], xy_full[sl], mmat_bf16))

    res = run_bass_kernel_spmd(
        nc, in_maps, core_ids=list(range(NCORES)), trace=trace, tmpdir=tmpdir
    )

    # ---- host-side exact linear functionals (float64) ----
    X64 = np.float64(X)
    xy64 = np.float64(xy_full)
    wvec = (C - 1) - 2.0 * np.arange(C, dtype=np.float64)
    sumd = (X64 @ wvec).sum()          # sum over rows of sum_{j<k}(x_j - x_k)
    xsum = X64.sum()
    xysum = xy64.sum()

    ls_eps = -math.log1p(math.exp(-EPS))
    log2 = math.log(2.0)

    sumln_tot = 0.0
    a_tot = 0.0
    mlnse_tot = 0.0
    for c in range(NCORES):
        parts = np.float64(res.results[c]["parts"])
        sumln_tot += parts[:, 0:8].sum()
        mlnse_tot += parts[:, 8:24].sum()   # lnse + mrow together
        a_tot += parts[:, 24].sum()

    t_sum = a_tot
    b_sum = a_tot - (xsum - C * xysum - N * C * EPS)

    ce_sum = mlnse_tot - xysum
    s_rest = a_tot + b_sum - sumd - 2.0 * sumln_tot + N * 101 * ls_eps

    loss_ce = ce_sum / N
    loss_bdc = (t_sum - N * log2) / ((C - 1) * N)
    loss_bec = -0.5 * s_rest / ((C - 1) * (C - 2) * N)
    loss = loss_ce + loss_bdc + loss_bec
    outs = tuple(
        np.float32(v) for v in (loss, loss_ce, loss_bdc, loss_bec)
    )
    return outs, res


def kernel(inputs, targets):
    X = np.ascontiguousarray(np.asarray(inputs, dtype=np.float32))
    tgt = np.asarray(targets).astype(np.int64)
    assert X.shape == (N, C), X.shape
    outs, _ = _run(X, tgt, trace=False)
    return outs
